# revision 30
# baseline (speedup 1.0000x reference)
"""Trainium2 Bass kernel for group-quant (fake int8, V=64) + Linear.

reference math (per row of x):
    absmax over feature-groups of 64 -> delta = max(2*absmax/254, 1e-5)
    xq = clip(round(x/delta), -127, 127) * delta      (fake quant)
    out = xq @ W.T + b

Sharding: 2-D — tokens 4-way x out-features 2-way across 8 cores.
Each core handles 2048 tokens x 2048 out-features; its W^T shard
([4096, 2048] fp16, pre-packed on host) is 128 KB/partition and stays
FULLY resident in SBUF: single phase, no W re-load, no x~^T spill
(the old 8-way token sharding streamed W twice and spilled x~^T,
starving the PE for the first ~180us).

x is cast to fp16 on the host: |x| <= ~6 so fp16's 10-bit mantissa
keeps the quant decisions almost always identical to fp32 (measured
end-to-end rel err 2.0e-3 vs the 2e-2 budget).  This halves x HBM
traffic to 16.8 MB/core — the first ~110us are DMA-fabric bound (the
W shard + x + XBAR transposes saturate the ~340 GB/s per-core fabric),
so x bytes trade 1:1 against how fast W can land.

Quant per 128-token tile (halves; quarters for tile 0 so the pipeline
primes in ~20us): group absmax/delta/recip on vector straight off the
fp16 tile; scale into an fp32 scratch with the fp16->fp32 upconvert
fused into the broadcast multiply, column-split vector/gpsimd; ONE
flat fused +/-1.5*2^23 round on vector covering the whole chunk (fp32
internal math rounds exactly to integer; the 2-op tensor_scalar is
~30x slower on gpsimd and partial-slice variants defeat Tile's region
tracking and serialize the engines); dequant fp32->fp16 back in place;
XBAR-transpose each half on the scalar queue.  NO ACT-engine compute
anywhere: the Activation queue only starts executing ~30us into every
kernel, so it carries nothing but transposes and output DMAs, and
PSUM is evacuated by DVE copies instead (gpsimd has no PSUM port).

Device schedule per core: a burst of dummy matmuls, gated on x tile
0's arrival, spans ~29-38us so the PE HAM clock-gate is at 8/8 just
as the first transpose (~33us) unblocks real matmuls — without it the
first real matmuls run at half clock.  Matmuls run k-outer/oc-inner
(4 per stationary, LDWEIGHTS fully hidden), accumulating into 4 PSUM
banks ping-ponged by token-tile parity.  The first four token tiles
run oc-pair-staggered segments so demand for W chunks 2/3 starts only
~62/69us in, pacing the sequential W stream (sync queue, nothing else
on it); the last tile runs oc-major to shrink the evac tail.
"""

import numpy as np

import concourse.bass as bass
import concourse.mybir as mybir
import concourse.tile as tile
from concourse.bass_utils import run_bass_kernel_spmd

N_CORES = 8
TP = 4                     # token-parallel ways
OP = 2                     # out-feature-parallel ways
MAGIC = 1.5 * 2.0**23      # fp32 round-to-nearest-even constant
QSCALE = 1.0 / 127.0       # 2/(qmax-qmin) with qmax=127, qmin=-127
DELTA_MIN = 1e-5


def _split_multiwait(nc):
    """This walrus build allows at most ONE sync wait per instruction
    ("Too many sync wait commands", CoreV3GenImpl setupSyncWait) and none
    on Drain. Tile freely attaches several waits to one instruction, so
    post-process: move excess waits onto single-wait NoOps inserted just
    before the instruction on the same engine queue (semantics identical —
    the queue stalls at the nop instead of at the instruction)."""
    nid = 0
    for fn in nc.m.functions:
        for bb in fn.blocks:
            insts = list(bb.instructions)
            out = []
            changed = False
            for inst in insts:
                si = inst.sync_info
                waits = list(si.on_wait) if si is not None and si.on_wait else []
                limit = 0 if type(inst).__name__ == "InstDrain" else 1
                if len(waits) > limit:
                    changed = True
                    keep = waits[len(waits) - limit :] if limit else []
                    for w in waits[: len(waits) - limit]:
                        nid += 1
                        out.append(
                            mybir.InstNoOp(
                                name=f"WSPLIT-{nid}",
                                engine=inst.engine,
                                bass_nofuse=True,
                                ins=[],
                                outs=[],
                                sync_info=mybir.SyncInfo(on_wait=[w], on_update=[]),
                            )
                        )
                    si.on_wait = keep
                out.append(inst)
            if changed:
                try:
                    bb.instructions = out
                except Exception:
                    bb.instructions[:] = out


def build(T=2048, K=4096, O=2048, V=64, GVH=12, wq_split=4, split=True,
          warm_mms=32, stag=4):
    f32, f16 = mybir.dt.float32, mybir.dt.float16
    P = 128
    G = K // V                 # quant groups per row (64)
    GH = G // 2                # groups per half (32)
    H = K // 2                 # cols per half (2048)
    KT = K // P                # contraction tiles (32)
    NT = T // P                # token tiles per core (16)
    OC = 512                   # oc chunk (psum bank width fp32)
    NOC = O // OC              # 4
    KQW = KT // wq_split       # k-tiles per W DMA quarter (8)

    nc = bass.Bass()
    x = nc.dram_tensor("x", [T, K], f16, kind="ExternalInput")
    wt = nc.dram_tensor("wt", [NOC, P, KT * OC], f16, kind="ExternalInput")
    out = nc.dram_tensor("out", [T, O], f32, kind="ExternalOutput")

    mult = mybir.AluOpType.mult
    amax_op = mybir.AluOpType.max

    with tile.TileContext(nc) as tc:
        with (
            tc.tile_pool(name="x", bufs=2) as pool_x,
            tc.tile_pool(name="q32", bufs=3) as pool_q,
            tc.tile_pool(name="st", bufs=2) as pool_s,
            tc.tile_pool(name="xt", bufs=4) as pool_xt,
            tc.tile_pool(name="w", bufs=1) as pool_w,
            tc.tile_pool(name="o", bufs=2) as pool_o,
            tc.tile_pool(name="ps", bufs=1, space="PSUM") as pool_ps,
        ):
            # ---- W shard loads: sync queue carries ONLY these ----
            def post_w(oc):
                wtile = pool_w.tile([P, KT, OC], f16, tag=f"w{oc}", name=f"w{oc}")
                for q in range(wq_split):
                    nc.sync.dma_start(
                        out=wtile[:, q * KQW : (q + 1) * KQW, :].rearrange(
                            "p k o -> p (k o)"
                        ),
                        in_=wt[oc][:, q * KQW * OC : (q + 1) * KQW * OC],
                    )
                return wtile

            wcur = [post_w(oc) for oc in range(NOC)]

            # ---- quant tile 0 DMA first (quarters, so its chain starts
            # the moment the first 0.25 MB lands) ----
            xq0 = pool_x.tile([P, K], f16, tag="x", name="x0")
            Q4 = K // 4
            for qq in range(4):
                nc.gpsimd.dma_start(
                    out=xq0[:, qq * Q4 : (qq + 1) * Q4],
                    in_=x[0:P, qq * Q4 : (qq + 1) * Q4],
                )

            # ---- PE warm-up: dummy matmuls gated on tile 0's first quant
            # chunk (copy creates the data dep), into a PSUM bank whose
            # first real use is late.  The first transpose can only reach
            # the PE ~33us in (the Activation hwdge queue starts ~32us
            # into every kernel), so the warm-up spans ~21-30us and the
            # HAM clock-gate is at 8/8 right when real matmuls start ----
            dummy = pool_w.tile([P, OC], f16, tag="warm", name="warm")

            def emit_warm(q32_gate):
                # gate on the (conservatively-tracked) x tile rather than
                # the chunk-0 scratch: the later trigger (~29us) makes the
                # warm-up span bridge exactly to the first transpose (~33us,
                # the Activation-queue start latency), so the PE never sits
                # idle past a HAM window before real matmuls begin
                nc.gpsimd.memset(dummy[:], 0.0)
                nc.gpsimd.tensor_copy(out=dummy[:, :P], in_=xq0[:, :P])
                wps = pool_ps.tile([P, OC], f32, tag="ps1_3", name="warmps")
                for i in range(warm_mms):
                    nc.tensor.matmul(
                        wps[:], dummy[:, :P], dummy[:],
                        start=(i == 0), stop=(i == warm_mms - 1),
                    )

            # ---- quant: per chunk (halves; quarters for tile 0) — stats
            # from fp16, scale into fp32 scratch, flat per-engine round,
            # dequant fp32->fp16 back into the x buffer, XBAR transpose.
            # NO ACT-engine instructions anywhere in the kernel: the ACT
            # table load blocks the Activation queue until ~40us, so the
            # scalar queue must stay pure-DMA (transposes + output) ----
            def rnd(eng, q32, c0, c1):   # exact fp32 RNE via +/-MAGIC, flat
                eng.tensor_scalar(
                    out=q32[:, c0:c1], in0=q32[:, c0:c1],
                    scalar1=MAGIC, scalar2=MAGIC,
                    op0=mybir.AluOpType.add, op1=mybir.AluOpType.subtract,
                )

            def emit_quant(t, xq_=None, nch=2, after_chunk0=None,
                           do_transpose=True):
                if xq_ is None:
                    xq_ = pool_x.tile([P, K], f16, tag="x", name=f"x{t}")
                    nc.gpsimd.dma_start(out=xq_[:], in_=x[t * P : (t + 1) * P, :])
                xts_t = pool_xt.tile([P, KT, P], f16, tag="xt", name=f"xts{t}")
                amax = pool_s.tile([P, G], f32, tag="amax", name=f"amax{t}")
                delta = pool_s.tile([P, G], f32, tag="delta", name=f"delta{t}")
                recip = pool_s.tile([P, G], f32, tag="recip", name=f"recip{t}")
                xr = xq_.rearrange("p (g v) -> p g v", v=V)
                CH = K // nch            # cols per chunk
                GC = G // nch            # groups per chunk
                KTC = KT // nch          # k-tiles per chunk
                GVC = (GVH * 2) // nch   # vector-side groups per chunk

                for h in range(nch):
                    g0 = h * GC
                    gs = slice(g0, g0 + GC)
                    # stats straight off the fp16 tile
                    nc.vector.tensor_reduce(
                        out=amax[:, gs], in_=xr[:, gs, :], axis=mybir.AxisListType.X,
                        op=amax_op, apply_absolute_value=True,
                    )
                    nc.vector.tensor_scalar(
                        out=delta[:, gs], in0=amax[:, gs],
                        scalar1=QSCALE, scalar2=DELTA_MIN, op0=mult,
                        op1=amax_op,
                    )
                    nc.vector.reciprocal(out=recip[:, gs], in_=delta[:, gs])

                    # column-split scale/dequant on vector+gpsimd; the
                    # fused +/-MAGIC round runs as ONE flat op on vector
                    # covering the whole chunk (the 2-op tensor_scalar is
                    # pathologically slow on gpsimd, ~15 ns/col, and drags
                    # concurrent vector ops with it)
                    q32 = pool_q.tile([P, H], f32, tag="q", name=f"q{t}_{h}")
                    qr = q32.rearrange("p (g v) -> p g v", v=V)

                    def rmul(eng, l0, l1):   # q32 = x16 * (1/delta)
                        eng.tensor_tensor(
                            out=qr[:, l0:l1, :], in0=xr[:, g0 + l0 : g0 + l1, :],
                            in1=recip[:, g0 + l0 : g0 + l1, None].to_broadcast(
                                (P, l1 - l0, V)), op=mult,
                        )

                    def dmul(eng, l0, l1):   # x16 = round(q32) * delta
                        eng.tensor_tensor(
                            out=xr[:, g0 + l0 : g0 + l1, :], in0=qr[:, l0:l1, :],
                            in1=delta[:, g0 + l0 : g0 + l1, None].to_broadcast(
                                (P, l1 - l0, V)), op=mult,
                        )

                    rmul(nc.vector, 0, GVC)
                    rmul(nc.gpsimd, GVC, GC)
                    rnd(nc.vector, q32, 0, GC * V)
                    dmul(nc.vector, 0, GVC)
                    dmul(nc.gpsimd, GVC, GC)
                    if do_transpose:
                        nc.scalar.dma_start_transpose(
                            xts_t[:, h * KTC : (h + 1) * KTC, :],
                            xq_[:, h * CH : (h + 1) * CH],
                        )
                    if h == 0 and after_chunk0 is not None:
                        after_chunk0(q32)
                return xts_t, xq_

            # ---- PSUM evac: DVE copy (gpsimd has no PSUM port), then
            # output DMA on the scalar queue (sync queue stays pure-W) ----
            def evac(t, oc, ps):
                ot = pool_o.tile([P, OC], f32, tag="o", name=f"ot{t}_{oc}")
                nc.vector.tensor_copy(out=ot[:], in_=ps[:])
                nc.scalar.dma_start(
                    out=out[t * P : (t + 1) * P, oc * OC : (oc + 1) * OC], in_=ot[:]
                )

            def emit_mm(t, xts_t, ocs, mode):
                if mode == "oc":
                    for oc in ocs:
                        ps = pool_ps.tile([P, OC], f32, tag=f"ps{t % 2}_{oc}",
                                          name=f"ps{t}_{oc}")
                        for kt in range(KT):
                            nc.tensor.matmul(
                                ps[:], xts_t[:, kt, :], wcur[oc][:, kt, :],
                                start=(kt == 0), stop=(kt == KT - 1),
                            )
                        evac(t, oc, ps)
                else:
                    pss = {
                        oc: pool_ps.tile([P, OC], f32, tag=f"ps{t % 2}_{oc}",
                                         name=f"ps{t}_{oc}")
                        for oc in ocs
                    }
                    for kt in range(KT):
                        for oc in ocs:
                            nc.tensor.matmul(
                                pss[oc][:], xts_t[:, kt, :], wcur[oc][:, kt, :],
                                start=(kt == 0), stop=(kt == KT - 1),
                            )
                    for oc in ocs:
                        evac(t, oc, pss[oc])

            # The first `stag` token tiles run oc-pair-staggered so demand
            # for W chunks 2/3 starts only ~62/69us in; "oc" mode on the
            # first segment of each pair lets matmuls start on partially-
            # arrived chunks.  Last tile oc-major to shrink the evac tail.
            ALL = list(range(NOC))
            segs = []
            for t in range(stag):
                segs.append((t, [0, 1], "oc" if t == 0 else "k"))
            for oc in (2, 3):
                for t in range(stag):
                    segs.append((t, [oc], "k"))
            n_phase1 = len(segs)
            for t in range(stag, NT - 1):
                segs.append((t, ALL, "k"))
            segs.append((NT - 1, ALL, "oc"))

            tiles = {0: emit_quant(0, xq0, nch=4, after_chunk0=emit_warm)}
            qnext = 1

            def emit_q_upto(n):
                nonlocal qnext
                while qnext < min(n, NT):
                    # tiles >= stag quant early (x-pool paced) but their
                    # transposes are DEFERRED: emitted inline they would
                    # wait on xts buffers released only by the [3]-phase
                    # segments and head-block the scalar ring
                    tiles[qnext] = emit_quant(qnext)
                    qnext += 1

            emit_q_upto(2)
            for si, (t, ocs, mode) in enumerate(segs):
                if t >= qnext:
                    emit_q_upto(t + 1)
                emit_mm(t, tiles[t][0], ocs, mode)
                if si < n_phase1:
                    # only tiles 0..stag-1 during the staggered phases: a
                    # later tile's transpose would wait on an xts buffer
                    # released only by the [3]-phase and head-block the
                    # scalar ring (out-DMAs -> evac copies -> vector)
                    emit_q_upto(min(3 + si, stag))
                else:
                    emit_q_upto(stag + 2 * (si - n_phase1 + 1))

    if split:
        _split_multiwait(nc)
    return nc


_CACHED = {}

# test-harness knobs (kernel() defaults are what the grader uses)
TRACE = False
LAST_RESULT = None
BUILD_KW = {}


def _get_nc(shape_key):
    if shape_key not in _CACHED:
        T, K, O = shape_key
        _CACHED[shape_key] = build(T=T, K=K, O=O, **BUILD_KW)
    return _CACHED[shape_key]


def pack_w(W: np.ndarray, OC: int = 512, P: int = 128) -> np.ndarray:
    # [out,in] -> W^T [in,out] fp16, packed [NOC, P, KT*OC] so each per-core
    # o-chunk W load is one fully contiguous DMA
    K, O = W.shape[1], W.shape[0]
    KT, NOC = K // P, O // OC
    wt = np.ascontiguousarray(W.T).astype(np.float16)         # [K, O]
    z = wt.reshape(KT, P, NOC, OC).transpose(2, 1, 0, 3)      # [NOC, P, KT, OC]
    return np.ascontiguousarray(z.reshape(NOC, P, KT * OC))


def kernel(x: np.ndarray, W: np.ndarray, b: np.ndarray) -> np.ndarray:
    global LAST_RESULT
    n, k = x.shape               # 8192, 4096
    o = W.shape[0]               # 4096
    assert n % TP == 0 and o % OP == 0
    tpc = n // TP                # 2048 tokens per core
    osh = o // OP                # 2048 out-features per core
    nc = _get_nc((tpc, k, osh))

    wtp = pack_w(W)              # [8, 128, 16384]
    ncs = osh // 512             # oc chunks per shard (4)
    xs = np.ascontiguousarray(x.astype(np.float16)).reshape(TP, tpc, k)
    in_maps = []
    for i in range(N_CORES):
        tb, ob = divmod(i, OP)
        in_maps.append(
            {"x": xs[tb], "wt": np.ascontiguousarray(wtp[ob * ncs : (ob + 1) * ncs])}
        )
    res = run_bass_kernel_spmd(nc, in_maps, list(range(N_CORES)), trace=TRACE)
    LAST_RESULT = res
    full = np.empty((n, o), np.float32)
    for i in range(N_CORES):
        tb, ob = divmod(i, OP)
        full[tb * tpc : (tb + 1) * tpc, ob * osh : (ob + 1) * osh] = (
            res.results[i]["out"]
        )
    full += b.astype(np.float32)[None, :]
    return full


# revision 33
# speedup vs baseline: 1.1462x; 1.1462x over previous
"""Trainium2 Bass kernel for group-quant (fake int8, V=64) + Linear.

reference math (per row of x):
    absmax over feature-groups of 64 -> delta = max(2*absmax/254, 1e-5)
    xq = clip(round(x/delta), -127, 127) * delta      (fake quant)
    out = xq @ W.T + b

Sharding: 2-D — tokens 4-way x out-features 2-way across 8 cores.
Each core handles 2048 tokens x 2048 out-features; its W^T shard
([4096, 2048] fp16, pre-packed on host) is 128 KB/partition and stays
FULLY resident in SBUF: single phase, no W re-load, no x~^T spill
(the old 8-way token sharding streamed W twice and spilled x~^T,
starving the PE for the first ~180us).

x is cast to fp16 on the host: |x| <= ~6 so fp16's 10-bit mantissa
keeps the quant decisions almost always identical to fp32 (measured
end-to-end rel err 2.0e-3 vs the 2e-2 budget).  This halves x HBM
traffic to 16.8 MB/core — the first ~110us are DMA-fabric bound (the
W shard + x + XBAR transposes saturate the ~340 GB/s per-core fabric),
so x bytes trade 1:1 against how fast W can land.

Quant per 128-token tile (halves; quarters for tile 0 so the pipeline
primes in ~20us): group absmax/delta/recip on vector straight off the
fp16 tile; scale into an fp32 scratch with the fp16->fp32 upconvert
fused into the broadcast multiply, column-split vector/gpsimd; ONE
flat fused +/-1.5*2^23 round on vector covering the whole chunk (fp32
internal math rounds exactly to integer; the 2-op tensor_scalar is
~30x slower on gpsimd and partial-slice variants defeat Tile's region
tracking and serialize the engines); dequant fp32->fp16 back in place;
XBAR-transpose each half on the scalar queue.  NO ACT-engine compute
anywhere: the Activation queue only starts executing ~30us into every
kernel, so it carries nothing but transposes and output DMAs, and
PSUM is evacuated by DVE copies instead (gpsimd has no PSUM port).

Device schedule per core: a burst of dummy matmuls, gated on x tile
0's arrival, spans ~29-38us so the PE HAM clock-gate is at 8/8 just
as the first transpose (~33us) unblocks real matmuls — without it the
first real matmuls run at half clock.  Matmuls run k-outer/oc-inner
(4 per stationary, LDWEIGHTS fully hidden), accumulating into 4 PSUM
banks ping-ponged by token-tile parity.  The first four token tiles
run oc-pair-staggered segments so demand for W chunks 2/3 starts only
~62/69us in, pacing the sequential W stream (sync queue, nothing else
on it); the last tile runs oc-major to shrink the evac tail.
"""

import numpy as np

import concourse.bass as bass
import concourse.mybir as mybir
import concourse.tile as tile
from concourse.bass_utils import run_bass_kernel_spmd

N_CORES = 8
TP = 4                     # token-parallel ways
OP = 2                     # out-feature-parallel ways
MAGIC = 1.5 * 2.0**23      # fp32 round-to-nearest-even constant
QSCALE = 1.0 / 127.0       # 2/(qmax-qmin) with qmax=127, qmin=-127
DELTA_MIN = 1e-5


def _split_multiwait(nc):
    """This walrus build allows at most ONE sync wait per instruction
    ("Too many sync wait commands", CoreV3GenImpl setupSyncWait) and none
    on Drain. Tile freely attaches several waits to one instruction, so
    post-process: move excess waits onto single-wait NoOps inserted just
    before the instruction on the same engine queue (semantics identical —
    the queue stalls at the nop instead of at the instruction)."""
    nid = 0
    for fn in nc.m.functions:
        for bb in fn.blocks:
            insts = list(bb.instructions)
            out = []
            changed = False
            for inst in insts:
                si = inst.sync_info
                waits = list(si.on_wait) if si is not None and si.on_wait else []
                limit = 0 if type(inst).__name__ == "InstDrain" else 1
                if len(waits) > limit:
                    changed = True
                    keep = waits[len(waits) - limit :] if limit else []
                    for w in waits[: len(waits) - limit]:
                        nid += 1
                        out.append(
                            mybir.InstNoOp(
                                name=f"WSPLIT-{nid}",
                                engine=inst.engine,
                                bass_nofuse=True,
                                ins=[],
                                outs=[],
                                sync_info=mybir.SyncInfo(on_wait=[w], on_update=[]),
                            )
                        )
                    si.on_wait = keep
                out.append(inst)
            if changed:
                try:
                    bb.instructions = out
                except Exception:
                    bb.instructions[:] = out


def build(T=2048, K=4096, O=2048, V=64, GVH=12, wq_split=4, split=True,
          warm_mms=40, stag=4):
    f32, f16 = mybir.dt.float32, mybir.dt.float16
    P = 128
    G = K // V                 # quant groups per row (64)
    GH = G // 2                # groups per half (32)
    H = K // 2                 # cols per half (2048)
    KT = K // P                # contraction tiles (32)
    NT = T // P                # token tiles per core (16)
    OC = 512                   # oc chunk (psum bank width fp32)
    NOC = O // OC              # 4
    KQW = KT // wq_split       # k-tiles per W DMA quarter (8)

    nc = bass.Bass()
    x = nc.dram_tensor("x", [T, K], f16, kind="ExternalInput")
    wt = nc.dram_tensor("wt", [NOC, P, KT * OC], f16, kind="ExternalInput")
    ident_d = nc.dram_tensor("ident", [P, P], f16, kind="ExternalInput")
    out = nc.dram_tensor("out", [T, O], f32, kind="ExternalOutput")

    mult = mybir.AluOpType.mult
    amax_op = mybir.AluOpType.max

    with tile.TileContext(nc) as tc:
        with (
            tc.tile_pool(name="x", bufs=2) as pool_x,
            tc.tile_pool(name="q32", bufs=3) as pool_q,
            tc.tile_pool(name="st", bufs=2) as pool_s,
            tc.tile_pool(name="xt", bufs=4) as pool_xt,
            tc.tile_pool(name="w", bufs=1) as pool_w,
            tc.tile_pool(name="o", bufs=2) as pool_o,
            tc.tile_pool(name="ps", bufs=1, space="PSUM") as pool_ps,
        ):
            # ---- W shard loads: sync queue carries ONLY these ----
            def post_w(oc):
                wtile = pool_w.tile([P, KT, OC], f16, tag=f"w{oc}", name=f"w{oc}")
                for q in range(wq_split):
                    nc.sync.dma_start(
                        out=wtile[:, q * KQW : (q + 1) * KQW, :].rearrange(
                            "p k o -> p (k o)"
                        ),
                        in_=wt[oc][:, q * KQW * OC : (q + 1) * KQW * OC],
                    )
                return wtile

            wcur = [post_w(oc) for oc in range(NOC)]

            identt = pool_w.tile([P, P], f16, tag="id", name="id")
            nc.gpsimd.dma_start(out=identt[:], in_=ident_d[:, :])

            # ---- quant tile 0 DMA first (quarters, so its chain starts
            # the moment the first 0.25 MB lands) ----
            xq0 = pool_x.tile([P, K], f16, tag="x", name="x0")
            Q4 = K // 4
            for qq in range(4):
                nc.gpsimd.dma_start(
                    out=xq0[:, qq * Q4 : (qq + 1) * Q4],
                    in_=x[0:P, qq * Q4 : (qq + 1) * Q4],
                )

            # ---- PE warm-up: dummy matmuls gated on tile 0's first quant
            # chunk (copy creates the data dep), into a PSUM bank whose
            # first real use is late.  The first transpose can only reach
            # the PE ~33us in (the Activation hwdge queue starts ~32us
            # into every kernel), so the warm-up spans ~21-30us and the
            # HAM clock-gate is at 8/8 right when real matmuls start ----
            dummy = pool_w.tile([P, OC], f16, tag="warm", name="warm")

            def emit_warm():
                # gate on the (conservatively-tracked) x tile rather than
                # the chunk-0 scratch: the later trigger (~29us) makes the
                # warm-up span bridge exactly to the first transpose (~33us,
                # the Activation-queue start latency), so the PE never sits
                # idle past a HAM window before real matmuls begin
                nc.gpsimd.memset(dummy[:], 0.0)
                nc.gpsimd.tensor_copy(out=dummy[:, :P], in_=xq0[:, :P])
                wps = pool_ps.tile([P, OC], f32, tag="ps1_3", name="warmps")
                for i in range(warm_mms):
                    nc.tensor.matmul(
                        wps[:], dummy[:, :P], dummy[:],
                        start=(i == 0), stop=(i == warm_mms - 1),
                    )

            # ---- quant: per chunk (halves; quarters for tile 0) — stats
            # from fp16, scale into fp32 scratch, flat per-engine round,
            # dequant fp32->fp16 back into the x buffer, XBAR transpose.
            # NO ACT-engine instructions anywhere in the kernel: the ACT
            # table load blocks the Activation queue until ~40us, so the
            # scalar queue must stay pure-DMA (transposes + output) ----
            def rnd(eng, q32, c0, c1):   # exact fp32 RNE via +/-MAGIC, flat
                eng.tensor_scalar(
                    out=q32[:, c0:c1], in0=q32[:, c0:c1],
                    scalar1=MAGIC, scalar2=MAGIC,
                    op0=mybir.AluOpType.add, op1=mybir.AluOpType.subtract,
                )

            def emit_quant(t, xq_=None, nch=2, after_chunk0=None,
                           pe_tr=False):
                if xq_ is None:
                    xq_ = pool_x.tile([P, K], f16, tag="x", name=f"x{t}")
                    nc.gpsimd.dma_start(out=xq_[:], in_=x[t * P : (t + 1) * P, :])
                xts_t = pool_xt.tile([P, KT, P], f16, tag="xt", name=f"xts{t}")
                amax = pool_s.tile([P, G], f32, tag="amax", name=f"amax{t}")
                delta = pool_s.tile([P, G], f32, tag="delta", name=f"delta{t}")
                recip = pool_s.tile([P, G], f32, tag="recip", name=f"recip{t}")
                xr = xq_.rearrange("p (g v) -> p g v", v=V)
                CH = K // nch            # cols per chunk
                GC = G // nch            # groups per chunk
                KTC = KT // nch          # k-tiles per chunk
                GVC = (GVH * 2) // nch   # vector-side groups per chunk

                for h in range(nch):
                    g0 = h * GC
                    gs = slice(g0, g0 + GC)
                    # stats straight off the fp16 tile
                    nc.vector.tensor_reduce(
                        out=amax[:, gs], in_=xr[:, gs, :], axis=mybir.AxisListType.X,
                        op=amax_op, apply_absolute_value=True,
                    )
                    nc.vector.tensor_scalar(
                        out=delta[:, gs], in0=amax[:, gs],
                        scalar1=QSCALE, scalar2=DELTA_MIN, op0=mult,
                        op1=amax_op,
                    )
                    nc.vector.reciprocal(out=recip[:, gs], in_=delta[:, gs])

                    # column-split scale/dequant on vector+gpsimd; the
                    # fused +/-MAGIC round runs as ONE flat op on vector
                    # covering the whole chunk (the 2-op tensor_scalar is
                    # pathologically slow on gpsimd, ~15 ns/col, and drags
                    # concurrent vector ops with it)
                    q32 = pool_q.tile([P, H], f32, tag="q", name=f"q{t}_{h}")
                    qr = q32.rearrange("p (g v) -> p g v", v=V)

                    def rmul(eng, l0, l1):   # q32 = x16 * (1/delta)
                        eng.tensor_tensor(
                            out=qr[:, l0:l1, :], in0=xr[:, g0 + l0 : g0 + l1, :],
                            in1=recip[:, g0 + l0 : g0 + l1, None].to_broadcast(
                                (P, l1 - l0, V)), op=mult,
                        )

                    def dmul(eng, l0, l1):   # x16 = round(q32) * delta
                        eng.tensor_tensor(
                            out=xr[:, g0 + l0 : g0 + l1, :], in0=qr[:, l0:l1, :],
                            in1=delta[:, g0 + l0 : g0 + l1, None].to_broadcast(
                                (P, l1 - l0, V)), op=mult,
                        )

                    rmul(nc.vector, 0, GVC)
                    rmul(nc.gpsimd, GVC, GC)
                    rnd(nc.vector, q32, 0, GC * V)
                    dmul(nc.vector, 0, GVC)
                    dmul(nc.gpsimd, GVC, GC)
                    if pe_tr:
                        # PE-mode transpose into PSUM banks whose first
                        # real use is seg (0,[2,3]) ~95us in
                        for g2 in range(CH // (4 * P)):
                            base = h * KTC + g2 * 4
                            trp = pool_ps.tile(
                                [P, OC], f16, tag=f"ps0_{2 + (base // 4) % 2}",
                                name=f"trp{base}",
                            )
                            for j in range(4):
                                kt = base + j
                                nc.tensor.transpose(
                                    trp[:, j * P : (j + 1) * P],
                                    xq_[:, kt * P : (kt + 1) * P], identt[:],
                                )
                            nc.vector.tensor_copy(
                                out=xts_t[:, base : base + 4, :], in_=trp[:]
                            )
                    else:
                        nc.scalar.dma_start_transpose(
                            xts_t[:, h * KTC : (h + 1) * KTC, :],
                            xq_[:, h * CH : (h + 1) * CH],
                        )
                    if h == 0 and after_chunk0 is not None:
                        after_chunk0()
                return xts_t, xq_

            # ---- PSUM evac: DVE copy (gpsimd has no PSUM port), then
            # output DMA on the scalar queue (sync queue stays pure-W) ----
            def evac(t, oc, ps):
                ot = pool_o.tile([P, OC], f32, tag="o", name=f"ot{t}_{oc}")
                nc.vector.tensor_copy(out=ot[:], in_=ps[:])
                nc.scalar.dma_start(
                    out=out[t * P : (t + 1) * P, oc * OC : (oc + 1) * OC], in_=ot[:]
                )

            def emit_mm(t, xts_t, ocs, mode):
                if mode == "oc":
                    for oc in ocs:
                        ps = pool_ps.tile([P, OC], f32, tag=f"ps{t % 2}_{oc}",
                                          name=f"ps{t}_{oc}")
                        for kt in range(KT):
                            nc.tensor.matmul(
                                ps[:], xts_t[:, kt, :], wcur[oc][:, kt, :],
                                start=(kt == 0), stop=(kt == KT - 1),
                            )
                        evac(t, oc, ps)
                else:
                    pss = {
                        oc: pool_ps.tile([P, OC], f32, tag=f"ps{t % 2}_{oc}",
                                         name=f"ps{t}_{oc}")
                        for oc in ocs
                    }
                    for kt in range(KT):
                        for oc in ocs:
                            nc.tensor.matmul(
                                pss[oc][:], xts_t[:, kt, :], wcur[oc][:, kt, :],
                                start=(kt == 0), stop=(kt == KT - 1),
                            )
                    for oc in ocs:
                        evac(t, oc, pss[oc])

            # The first `stag` token tiles run oc-pair-staggered so demand
            # for W chunks 2/3 starts only ~62/69us in; "oc" mode on the
            # first segment of each pair lets matmuls start on partially-
            # arrived chunks.  Last tile oc-major to shrink the evac tail.
            ALL = list(range(NOC))
            segs = []
            for pair in ([0, 1], [2, 3]):
                for t in range(stag):
                    segs.append((t, pair, "oc" if t == 0 else "k"))
            for t in range(stag, NT - 1):
                segs.append((t, ALL, "k"))
            segs.append((NT - 1, ALL, "oc"))

            emit_warm()
            tiles = {0: emit_quant(0, xq0, nch=4, pe_tr=True)}
            qnext = 1

            def emit_q_upto(n):
                nonlocal qnext
                while qnext < min(n, NT):
                    # tiles >= stag quant early (x-pool paced) but their
                    # transposes are DEFERRED: emitted inline they would
                    # wait on xts buffers released only by the [3]-phase
                    # segments and head-block the scalar ring
                    tiles[qnext] = emit_quant(qnext)
                    qnext += 1

            emit_q_upto(2)
            for si, (t, ocs, mode) in enumerate(segs):
                if t >= qnext:
                    emit_q_upto(t + 1)
                emit_mm(t, tiles[t][0], ocs, mode)
                emit_q_upto(3 + si)

    if split:
        _split_multiwait(nc)
    return nc


_CACHED = {}

# test-harness knobs (kernel() defaults are what the grader uses)
TRACE = False
LAST_RESULT = None
BUILD_KW = {}


def _get_nc(shape_key):
    if shape_key not in _CACHED:
        T, K, O = shape_key
        _CACHED[shape_key] = build(T=T, K=K, O=O, **BUILD_KW)
    return _CACHED[shape_key]


def pack_w(W: np.ndarray, OC: int = 512, P: int = 128) -> np.ndarray:
    # [out,in] -> W^T [in,out] fp16, packed [NOC, P, KT*OC] so each per-core
    # o-chunk W load is one fully contiguous DMA
    K, O = W.shape[1], W.shape[0]
    KT, NOC = K // P, O // OC
    wt = np.ascontiguousarray(W.T).astype(np.float16)         # [K, O]
    z = wt.reshape(KT, P, NOC, OC).transpose(2, 1, 0, 3)      # [NOC, P, KT, OC]
    return np.ascontiguousarray(z.reshape(NOC, P, KT * OC))


def kernel(x: np.ndarray, W: np.ndarray, b: np.ndarray) -> np.ndarray:
    global LAST_RESULT
    n, k = x.shape               # 8192, 4096
    o = W.shape[0]               # 4096
    assert n % TP == 0 and o % OP == 0
    tpc = n // TP                # 2048 tokens per core
    osh = o // OP                # 2048 out-features per core
    nc = _get_nc((tpc, k, osh))

    wtp = pack_w(W)              # [8, 128, 16384]
    ident = np.eye(128, dtype=np.float16)
    ncs = osh // 512             # oc chunks per shard (4)
    xs = np.ascontiguousarray(x.astype(np.float16)).reshape(TP, tpc, k)
    in_maps = []
    for i in range(N_CORES):
        tb, ob = divmod(i, OP)
        in_maps.append(
            {"x": xs[tb], "wt": np.ascontiguousarray(wtp[ob * ncs : (ob + 1) * ncs]),
             "ident": ident}
        )
    res = run_bass_kernel_spmd(nc, in_maps, list(range(N_CORES)), trace=TRACE)
    LAST_RESULT = res
    full = np.empty((n, o), np.float32)
    for i in range(N_CORES):
        tb, ob = divmod(i, OP)
        full[tb * tpc : (tb + 1) * tpc, ob * osh : (ob + 1) * osh] = (
            res.results[i]["out"]
        )
    full += b.astype(np.float32)[None, :]
    return full


# revision 36
# speedup vs baseline: 1.1471x; 1.0008x over previous
"""Trainium2 Bass kernel for group-quant (fake int8, V=64) + Linear.

reference math (per row of x):
    absmax over feature-groups of 64 -> delta = max(2*absmax/254, 1e-5)
    xq = clip(round(x/delta), -127, 127) * delta      (fake quant)
    out = xq @ W.T + b

Sharding: 2-D — tokens 4-way x out-features 2-way across 8 cores.
Each core handles 2048 tokens x 2048 out-features; its W^T shard
([4096, 2048] fp16, pre-packed on host) is 128 KB/partition and stays
FULLY resident in SBUF: single phase, no W re-load, no x~^T spill
(the old 8-way token sharding streamed W twice and spilled x~^T,
starving the PE for the first ~180us).

x is cast to fp16 on the host: |x| <= ~6 so fp16's 10-bit mantissa
keeps the quant decisions almost always identical to fp32 (measured
end-to-end rel err 2.0e-3 vs the 2e-2 budget).  This halves x HBM
traffic to 16.8 MB/core — the first ~110us are DMA-fabric bound (the
W shard + x + XBAR transposes saturate the ~340 GB/s per-core fabric),
so x bytes trade 1:1 against how fast W can land.

Quant per 128-token tile (halves; quarters for tile 0 so the pipeline
primes in ~20us): group absmax/delta/recip on vector straight off the
fp16 tile; scale into an fp32 scratch with the fp16->fp32 upconvert
fused into the broadcast multiply, column-split vector/gpsimd; ONE
flat fused +/-1.5*2^23 round on vector covering the whole chunk (fp32
internal math rounds exactly to integer; the 2-op tensor_scalar is
~30x slower on gpsimd and partial-slice variants defeat Tile's region
tracking and serialize the engines); dequant fp32->fp16 back in place;
XBAR-transpose each half on the scalar queue.  NO ACT-engine compute
anywhere: the Activation queue only starts executing ~30us into every
kernel, so it carries nothing but transposes and output DMAs, and
PSUM is evacuated by DVE copies instead (gpsimd has no PSUM port).
TILE 0 is transposed on the PE instead (transpose-mode matmuls against
a host-supplied identity, evacuated 4 k-tiles per DVE copy from PSUM
banks that real matmuls first touch ~95us in), sidestepping the
Activation-queue latency so real matmuls start ~27us instead of ~38.

Device schedule per core: a burst of dummy matmuls, gated on x tile
0's DMA arrival, bridges ~17-26us so the PE HAM clock-gate is at 8/8
when the PE transposes and first real matmuls begin — without it the
first real matmuls run at half clock.  Matmuls run k-outer/oc-inner
(4 per stationary, LDWEIGHTS fully hidden), accumulating into 4 PSUM
banks ping-ponged by token-tile parity.  The first four token tiles
run oc-pair-staggered segments so demand for W chunks 2/3 starts only
~62/69us in, pacing the sequential W stream (sync queue, nothing else
on it); the last tile runs oc-major to shrink the evac tail.
"""

import numpy as np

import concourse.bass as bass
import concourse.mybir as mybir
import concourse.tile as tile
from concourse.bass_utils import run_bass_kernel_spmd

N_CORES = 8
TP = 4                     # token-parallel ways
OP = 2                     # out-feature-parallel ways
MAGIC = 1.5 * 2.0**23      # fp32 round-to-nearest-even constant
QSCALE = 1.0 / 127.0       # 2/(qmax-qmin) with qmax=127, qmin=-127
DELTA_MIN = 1e-5


def _split_multiwait(nc):
    """This walrus build allows at most ONE sync wait per instruction
    ("Too many sync wait commands", CoreV3GenImpl setupSyncWait) and none
    on Drain. Tile freely attaches several waits to one instruction, so
    post-process: move excess waits onto single-wait NoOps inserted just
    before the instruction on the same engine queue (semantics identical —
    the queue stalls at the nop instead of at the instruction)."""
    nid = 0
    for fn in nc.m.functions:
        for bb in fn.blocks:
            insts = list(bb.instructions)
            out = []
            changed = False
            for inst in insts:
                si = inst.sync_info
                waits = list(si.on_wait) if si is not None and si.on_wait else []
                limit = 0 if type(inst).__name__ == "InstDrain" else 1
                if len(waits) > limit:
                    changed = True
                    keep = waits[len(waits) - limit :] if limit else []
                    for w in waits[: len(waits) - limit]:
                        nid += 1
                        out.append(
                            mybir.InstNoOp(
                                name=f"WSPLIT-{nid}",
                                engine=inst.engine,
                                bass_nofuse=True,
                                ins=[],
                                outs=[],
                                sync_info=mybir.SyncInfo(on_wait=[w], on_update=[]),
                            )
                        )
                    si.on_wait = keep
                out.append(inst)
            if changed:
                try:
                    bb.instructions = out
                except Exception:
                    bb.instructions[:] = out


def build(T=2048, K=4096, O=2048, V=64, GVH=12, wq_split=4, split=True,
          warm_mms=40, stag=4):
    f32, f16 = mybir.dt.float32, mybir.dt.float16
    P = 128
    G = K // V                 # quant groups per row (64)
    GH = G // 2                # groups per half (32)
    H = K // 2                 # cols per half (2048)
    KT = K // P                # contraction tiles (32)
    NT = T // P                # token tiles per core (16)
    OC = 512                   # oc chunk (psum bank width fp32)
    NOC = O // OC              # 4
    KQW = KT // wq_split       # k-tiles per W DMA quarter (8)

    nc = bass.Bass()
    x = nc.dram_tensor("x", [T, K], f16, kind="ExternalInput")
    wt = nc.dram_tensor("wt", [NOC, P, KT * OC], f16, kind="ExternalInput")
    ident_d = nc.dram_tensor("ident", [P, P], f16, kind="ExternalInput")
    out = nc.dram_tensor("out", [T, O], f32, kind="ExternalOutput")

    mult = mybir.AluOpType.mult
    amax_op = mybir.AluOpType.max

    with tile.TileContext(nc) as tc:
        with (
            tc.tile_pool(name="x", bufs=2) as pool_x,
            tc.tile_pool(name="q32", bufs=3) as pool_q,
            tc.tile_pool(name="st", bufs=2) as pool_s,
            tc.tile_pool(name="xt", bufs=4) as pool_xt,
            tc.tile_pool(name="w", bufs=1) as pool_w,
            tc.tile_pool(name="o", bufs=2) as pool_o,
            tc.tile_pool(name="ps", bufs=1, space="PSUM") as pool_ps,
        ):
            # ---- W shard loads: chunks 0-2 on the sync queue (which
            # carries ONLY W); chunk 3 rides the scalar ring — idle until
            # ~50us now that tile 0 transposes on the PE — emitted after
            # tile 3's XBAR transposes so it lands ~95us instead of ~113 ----
            def post_w(oc, eng=None):
                eng = eng or nc.sync
                wtile = pool_w.tile([P, KT, OC], f16, tag=f"w{oc}", name=f"w{oc}")
                for q in range(wq_split):
                    eng.dma_start(
                        out=wtile[:, q * KQW : (q + 1) * KQW, :].rearrange(
                            "p k o -> p (k o)"
                        ),
                        in_=wt[oc][:, q * KQW * OC : (q + 1) * KQW * OC],
                    )
                return wtile

            wcur = [post_w(oc) for oc in range(NOC - 1)] + [None]

            identt = pool_w.tile([P, P], f16, tag="id", name="id")
            nc.gpsimd.dma_start(out=identt[:], in_=ident_d[:, :])

            # ---- quant tile 0 DMA first (quarters, so its chain starts
            # the moment the first 0.25 MB lands) ----
            xq0 = pool_x.tile([P, K], f16, tag="x", name="x0")
            Q4 = K // 4
            for qq in range(4):
                nc.gpsimd.dma_start(
                    out=xq0[:, qq * Q4 : (qq + 1) * Q4],
                    in_=x[0:P, qq * Q4 : (qq + 1) * Q4],
                )

            # ---- PE warm-up: dummy matmuls gated on tile 0's first quant
            # chunk (copy creates the data dep), into a PSUM bank whose
            # first real use is late.  The first transpose can only reach
            # the PE ~33us in (the Activation hwdge queue starts ~32us
            # into every kernel), so the warm-up spans ~21-30us and the
            # HAM clock-gate is at 8/8 right when real matmuls start ----
            dummy = pool_w.tile([P, OC], f16, tag="warm", name="warm")

            def emit_warm():
                # gate on the (conservatively-tracked) x tile rather than
                # the chunk-0 scratch: the later trigger (~29us) makes the
                # warm-up span bridge exactly to the first transpose (~33us,
                # the Activation-queue start latency), so the PE never sits
                # idle past a HAM window before real matmuls begin
                nc.gpsimd.memset(dummy[:], 0.0)
                nc.gpsimd.tensor_copy(out=dummy[:, :P], in_=xq0[:, :P])
                wps = pool_ps.tile([P, OC], f32, tag="ps1_3", name="warmps")
                for i in range(warm_mms):
                    nc.tensor.matmul(
                        wps[:], dummy[:, :P], dummy[:],
                        start=(i == 0), stop=(i == warm_mms - 1),
                    )

            # ---- quant: per chunk (halves; quarters for tile 0) — stats
            # from fp16, scale into fp32 scratch, flat per-engine round,
            # dequant fp32->fp16 back into the x buffer, XBAR transpose.
            # NO ACT-engine instructions anywhere in the kernel: the ACT
            # table load blocks the Activation queue until ~40us, so the
            # scalar queue must stay pure-DMA (transposes + output) ----
            def rnd(eng, q32, c0, c1):   # exact fp32 RNE via +/-MAGIC, flat
                eng.tensor_scalar(
                    out=q32[:, c0:c1], in0=q32[:, c0:c1],
                    scalar1=MAGIC, scalar2=MAGIC,
                    op0=mybir.AluOpType.add, op1=mybir.AluOpType.subtract,
                )

            def emit_quant(t, xq_=None, nch=2, after_chunk0=None,
                           pe_tr=False):
                if xq_ is None:
                    xq_ = pool_x.tile([P, K], f16, tag="x", name=f"x{t}")
                    nc.gpsimd.dma_start(out=xq_[:], in_=x[t * P : (t + 1) * P, :])
                xts_t = pool_xt.tile([P, KT, P], f16, tag="xt", name=f"xts{t}")
                amax = pool_s.tile([P, G], f32, tag="amax", name=f"amax{t}")
                delta = pool_s.tile([P, G], f32, tag="delta", name=f"delta{t}")
                recip = pool_s.tile([P, G], f32, tag="recip", name=f"recip{t}")
                xr = xq_.rearrange("p (g v) -> p g v", v=V)
                CH = K // nch            # cols per chunk
                GC = G // nch            # groups per chunk
                KTC = KT // nch          # k-tiles per chunk
                GVC = (GVH * 2) // nch   # vector-side groups per chunk

                for h in range(nch):
                    g0 = h * GC
                    gs = slice(g0, g0 + GC)
                    # stats straight off the fp16 tile
                    nc.vector.tensor_reduce(
                        out=amax[:, gs], in_=xr[:, gs, :], axis=mybir.AxisListType.X,
                        op=amax_op, apply_absolute_value=True,
                    )
                    nc.vector.tensor_scalar(
                        out=delta[:, gs], in0=amax[:, gs],
                        scalar1=QSCALE, scalar2=DELTA_MIN, op0=mult,
                        op1=amax_op,
                    )
                    nc.vector.reciprocal(out=recip[:, gs], in_=delta[:, gs])

                    # column-split scale/dequant on vector+gpsimd; the
                    # fused +/-MAGIC round runs as ONE flat op on vector
                    # covering the whole chunk (the 2-op tensor_scalar is
                    # pathologically slow on gpsimd, ~15 ns/col, and drags
                    # concurrent vector ops with it)
                    q32 = pool_q.tile([P, H], f32, tag="q", name=f"q{t}_{h}")
                    qr = q32.rearrange("p (g v) -> p g v", v=V)

                    def rmul(eng, l0, l1):   # q32 = x16 * (1/delta)
                        eng.tensor_tensor(
                            out=qr[:, l0:l1, :], in0=xr[:, g0 + l0 : g0 + l1, :],
                            in1=recip[:, g0 + l0 : g0 + l1, None].to_broadcast(
                                (P, l1 - l0, V)), op=mult,
                        )

                    def dmul(eng, l0, l1):   # x16 = round(q32) * delta
                        eng.tensor_tensor(
                            out=xr[:, g0 + l0 : g0 + l1, :], in0=qr[:, l0:l1, :],
                            in1=delta[:, g0 + l0 : g0 + l1, None].to_broadcast(
                                (P, l1 - l0, V)), op=mult,
                        )

                    rmul(nc.vector, 0, GVC)
                    rmul(nc.gpsimd, GVC, GC)
                    rnd(nc.vector, q32, 0, GC * V)
                    dmul(nc.vector, 0, GVC)
                    dmul(nc.gpsimd, GVC, GC)
                    if pe_tr:
                        # PE-mode transpose into PSUM banks whose first
                        # real use is seg (0,[2,3]) ~95us in
                        for g2 in range(CH // (4 * P)):
                            base = h * KTC + g2 * 4
                            trp = pool_ps.tile(
                                [P, OC], f16, tag=f"ps0_{2 + (base // 4) % 2}",
                                name=f"trp{base}",
                            )
                            for j in range(4):
                                kt = base + j
                                nc.tensor.transpose(
                                    trp[:, j * P : (j + 1) * P],
                                    xq_[:, kt * P : (kt + 1) * P], identt[:],
                                )
                            nc.vector.tensor_copy(
                                out=xts_t[:, base : base + 4, :], in_=trp[:]
                            )
                    else:
                        nc.scalar.dma_start_transpose(
                            xts_t[:, h * KTC : (h + 1) * KTC, :],
                            xq_[:, h * CH : (h + 1) * CH],
                        )
                    if h == 0 and after_chunk0 is not None:
                        after_chunk0()
                return xts_t, xq_

            # ---- PSUM evac: DVE copy (gpsimd has no PSUM port), then
            # output DMA on the scalar queue (sync queue stays pure-W) ----
            def evac(t, oc, ps):
                ot = pool_o.tile([P, OC], f32, tag="o", name=f"ot{t}_{oc}")
                nc.vector.tensor_copy(out=ot[:], in_=ps[:])
                nc.scalar.dma_start(
                    out=out[t * P : (t + 1) * P, oc * OC : (oc + 1) * OC], in_=ot[:]
                )

            def emit_mm(t, xts_t, ocs, mode):
                if mode == "oc":
                    for oc in ocs:
                        ps = pool_ps.tile([P, OC], f32, tag=f"ps{t % 2}_{oc}",
                                          name=f"ps{t}_{oc}")
                        for kt in range(KT):
                            nc.tensor.matmul(
                                ps[:], xts_t[:, kt, :], wcur[oc][:, kt, :],
                                start=(kt == 0), stop=(kt == KT - 1),
                            )
                        evac(t, oc, ps)
                else:
                    pss = {
                        oc: pool_ps.tile([P, OC], f32, tag=f"ps{t % 2}_{oc}",
                                         name=f"ps{t}_{oc}")
                        for oc in ocs
                    }
                    for kt in range(KT):
                        for oc in ocs:
                            nc.tensor.matmul(
                                pss[oc][:], xts_t[:, kt, :], wcur[oc][:, kt, :],
                                start=(kt == 0), stop=(kt == KT - 1),
                            )
                    for oc in ocs:
                        evac(t, oc, pss[oc])

            # The first `stag` token tiles run oc-pair-staggered so demand
            # for W chunks 2/3 starts only ~62/69us in; "oc" mode on the
            # first segment of each pair lets matmuls start on partially-
            # arrived chunks.  Last tile oc-major to shrink the evac tail.
            ALL = list(range(NOC))
            segs = []
            for pair in ([0, 1], [2, 3]):
                for t in range(stag):
                    segs.append((t, pair, "oc" if t == 0 else "k"))
            for t in range(stag, NT - 1):
                segs.append((t, ALL, "k"))
            segs.append((NT - 1, ALL, "oc"))

            emit_warm()
            tiles = {0: emit_quant(0, xq0, nch=4, pe_tr=True)}
            qnext = 1

            def emit_q_upto(n):
                nonlocal qnext
                while qnext < min(n, NT):
                    tiles[qnext] = emit_quant(qnext)
                    qnext += 1
                    if qnext == stag and wcur[NOC - 1] is None:
                        # last W chunk rides the scalar ring behind tile
                        # 3's transposes; it lands ~95us vs ~113 on sync
                        wcur[NOC - 1] = post_w(NOC - 1, nc.scalar)

            emit_q_upto(2)
            for si, (t, ocs, mode) in enumerate(segs):
                if t >= qnext:
                    emit_q_upto(t + 1)
                emit_mm(t, tiles[t][0], ocs, mode)
                emit_q_upto(3 + si)

    if split:
        _split_multiwait(nc)
    return nc


_CACHED = {}

# test-harness knobs (kernel() defaults are what the grader uses)
TRACE = False
LAST_RESULT = None
BUILD_KW = {}


def _get_nc(shape_key):
    if shape_key not in _CACHED:
        T, K, O = shape_key
        _CACHED[shape_key] = build(T=T, K=K, O=O, **BUILD_KW)
    return _CACHED[shape_key]


def pack_w(W: np.ndarray, OC: int = 512, P: int = 128) -> np.ndarray:
    # [out,in] -> W^T [in,out] fp16, packed [NOC, P, KT*OC] so each per-core
    # o-chunk W load is one fully contiguous DMA
    K, O = W.shape[1], W.shape[0]
    KT, NOC = K // P, O // OC
    wt = np.ascontiguousarray(W.T).astype(np.float16)         # [K, O]
    z = wt.reshape(KT, P, NOC, OC).transpose(2, 1, 0, 3)      # [NOC, P, KT, OC]
    return np.ascontiguousarray(z.reshape(NOC, P, KT * OC))


def kernel(x: np.ndarray, W: np.ndarray, b: np.ndarray) -> np.ndarray:
    global LAST_RESULT
    n, k = x.shape               # 8192, 4096
    o = W.shape[0]               # 4096
    assert n % TP == 0 and o % OP == 0
    tpc = n // TP                # 2048 tokens per core
    osh = o // OP                # 2048 out-features per core
    nc = _get_nc((tpc, k, osh))

    wtp = pack_w(W)              # [8, 128, 16384]
    ident = np.eye(128, dtype=np.float16)
    ncs = osh // 512             # oc chunks per shard (4)
    xs = np.ascontiguousarray(x.astype(np.float16)).reshape(TP, tpc, k)
    in_maps = []
    for i in range(N_CORES):
        tb, ob = divmod(i, OP)
        in_maps.append(
            {"x": xs[tb], "wt": np.ascontiguousarray(wtp[ob * ncs : (ob + 1) * ncs]),
             "ident": ident}
        )
    res = run_bass_kernel_spmd(nc, in_maps, list(range(N_CORES)), trace=TRACE)
    LAST_RESULT = res
    full = np.empty((n, o), np.float32)
    for i in range(N_CORES):
        tb, ob = divmod(i, OP)
        full[tb * tpc : (tb + 1) * tpc, ob * osh : (ob + 1) * osh] = (
            res.results[i]["out"]
        )
    full += b.astype(np.float32)[None, :]
    return full
